# revision 1
# baseline (speedup 1.0000x reference)
"""Multi-head causal attention (B=4, S=2048, H=1024, NH=16) on 8 trn2 cores.

Head-sharded tensor parallelism: core i computes heads {2i, 2i+1}.  Each core
runs projections for its 2 heads (fp32r matmuls), causal flash-style attention
in a transposed orientation (scores S^T[k,q] so the P@V contraction needs no
transpose of P), and a partial output projection over its 128 channels.  The
8 partial outputs are summed on the host (the tensor-parallel all-reduce),
plus the output bias.
"""
import numpy as np

import concourse.bacc as bacc
import concourse.tile as tile
from concourse import mybir
from concourse.bass_utils import run_bass_kernel_spmd

F32 = mybir.dt.float32
F32R = mybir.dt.float32r
AF = mybir.ActivationFunctionType

B, S, H, NH = 4, 2048, 1024, 16
HD = H // NH            # 64
NCORES = 8
HPC = NH // NCORES      # 2 heads per core
C = HPC * HD            # 128 channels per core
SCALE = 1.0 / np.sqrt(HD)

QT_W = 256              # q-tile width (columns of S^T tiles)
KC = 128                # k-chunk (contraction tile for P@V)
N_QT = S // QT_W        # 8
N_KC = S // KC          # 16
N_HC = H // 128         # 8 contraction chunks for projections
N_ST = 4                # s-tiles of 512 for projections

_CACHE = {}
PHASES = ("proj", "vtrans", "attn", "oproj")
PROJ_PRIO = 0


def _build_nc():
    nc = bacc.Bacc(name="mha_tp")
    xt_d = nc.dram_tensor("xt", [B, H, S], F32R, kind="ExternalInput")
    wq_d = nc.dram_tensor("wqt", [H, C], F32R, kind="ExternalInput")
    wk_d = nc.dram_tensor("wkt", [H, C], F32R, kind="ExternalInput")
    wv_d = nc.dram_tensor("wvt", [H, C], F32R, kind="ExternalInput")
    wo_d = nc.dram_tensor("wot", [C, H], F32R, kind="ExternalInput")
    bq_d = nc.dram_tensor("bq", [C, 1], F32, kind="ExternalInput")
    bk_d = nc.dram_tensor("bk", [C, 1], F32, kind="ExternalInput")
    bv_d = nc.dram_tensor("bv", [C, 1], F32, kind="ExternalInput")
    mk_d = nc.dram_tensor("maskbuf", [128, 896], F32R, kind="ExternalInput")
    id_d = nc.dram_tensor("ident", [128, 128], F32, kind="ExternalInput")
    on_d = nc.dram_tensor("ones16", [128, N_KC], F32R, kind="ExternalInput")
    out_d = nc.dram_tensor("out", [B, S, H], F32, kind="ExternalOutput")

    with tile.TileContext(nc) as tc:
        with (
            tc.tile_pool(name="const", bufs=1) as cp,
            tc.tile_pool(name="big", bufs=2) as bp,
            tc.tile_pool(name="work", bufs=2) as wp,
            tc.tile_pool(name="xs", bufs=12) as xp,
            tc.tile_pool(name="ps", bufs=1, space="PSUM") as ps,
            tc.tile_pool(name="psmix", bufs=2, space="PSUM") as pm,
        ):
            # ---- constants ----
            wq_s = cp.tile([128, H], F32R)
            wk_s = cp.tile([128, H], F32R)
            wv_s = cp.tile([128, H], F32R)
            wo_s = cp.tile([128, H], F32R)
            mk_s = cp.tile([128, 896], F32R)
            id_s = cp.tile([128, 128], F32)
            on_s = cp.tile([128, N_KC], F32R)
            bq_s = cp.tile([C, 1], F32)
            bk_s = cp.tile([C, 1], F32)
            bv_s = cp.tile([C, 1], F32)
            for w_s, w_d in ((wq_s, wq_d), (wk_s, wk_d), (wv_s, wv_d)):
                nc.scalar.dma_start(
                    w_s.rearrange("p (c d) -> p c d", d=128),
                    w_d.ap().rearrange("(c p) d -> p c d", p=128))
            nc.scalar.dma_start(wo_s[:], wo_d.ap())
            nc.scalar.dma_start(mk_s[:], mk_d.ap())
            nc.scalar.dma_start(id_s[:], id_d.ap())
            nc.scalar.dma_start(on_s[:], on_d.ap())
            nc.scalar.dma_start(bq_s[:], bq_d.ap())
            nc.scalar.dma_start(bk_s[:], bk_d.ap())
            nc.scalar.dma_start(bv_s[:], bv_d.ap())

            tiles = {}

            def emit_proj(b, halves=(0, 1)):
                # ---- projections: QT/KT [128, S] f32r, VT [128, S] f32 ----
                if b not in tiles:
                    qt = bp.tile([128, S], F32R, tag="qt", name=f"qt{b}")
                    kt = bp.tile([128, S], F32R, tag="kt", name=f"kt{b}")
                    vt = bp.tile([128, S], F32, tag="vt", name=f"vt{b}", bufs=1)
                    tiles[b] = {"qt": qt, "kt": kt, "vt": vt}
                qt, kt, vt = tiles[b]["qt"], tiles[b]["kt"], tiles[b]["vt"]
                if True:
                  for half in halves if "proj" in PHASES else []:
                    xts = []
                    for hc in range(N_HC):
                        hsl = slice(hc * 128, (hc + 1) * 128)
                        xt_t = xp.tile([128, 1024], F32R, tag="xt",
                                       name=f"x{b}_{half}_{hc}")
                        nc.sync.dma_start(
                            xt_t[:], xt_d.ap()[b, hsl, half * 1024:(half + 1) * 1024])
                        xts.append(xt_t)
                    for sth in range(2):
                        st = half * 2 + sth
                        ssl = slice(st * 512, (st + 1) * 512)
                        # sequential Q/K/V passes over resident x^T chunks: 2
                        # PSUM slots suffice (pipeline pass i+1 against copy i)
                        for w_s, bias, dst, pnm in ((wq_s, bq_s, qt, "q"),
                                                    (wk_s, bk_s, kt, "k"),
                                                    (wv_s, bv_s, vt, "v")):
                            pp = pm.tile([128, 512], F32, tag="mix",
                                         name=f"pp{pnm}{b}_{st}")
                            for hc in range(N_HC):
                                nc.tensor.matmul(
                                    pp[:], w_s[:, hc * 128:(hc + 1) * 128],
                                    xts[hc][:, sth * 512:(sth + 1) * 512],
                                    start=(hc == 0), stop=(hc == N_HC - 1))
                            nc.vector.tensor_scalar_add(dst[:, ssl], pp[:], bias[:])

            def emit_vtrans(b):
                # ---- V transpose: vn_h [128, 16*65] (ones col at 64 of each 65) ----
                vt = tiles[b]["vt"]
                vna = bp.tile([128, N_KC * (HD + 1)], F32R, tag="vna", name=f"vna{b}")
                vnb = bp.tile([128, N_KC * (HD + 1)], F32R, tag="vnb", name=f"vnb{b}")
                tiles[b]["vna"], tiles[b]["vnb"] = vna, vnb
                for h, vn in ((0, vna), (1, vnb)):
                    vn3 = vn.rearrange("p (c e) -> p c e", e=HD + 1)
                    nc.sync.dma_start(vn3[:, :, HD], on_d.ap())
                for c in range(N_KC) if "vtrans" in PHASES else []:
                    tp = pm.tile([128, 128], F32, tag="mix", name=f"tp{b}_{c}")
                    nc.tensor.transpose(tp[:], vt[:, c * 128:(c + 1) * 128], id_s[:])
                    nc.any.tensor_copy(vna[:, c * (HD + 1): c * (HD + 1) + HD],
                                       tp[:, 0:HD])
                    nc.any.tensor_copy(vnb[:, c * (HD + 1): c * (HD + 1) + HD],
                                       tp[:, HD:2 * HD])

            def emit_attn(b, jlo=0, jhi=N_QT):
                # ---- attention (transposed scores), both heads interleaved ----
                qt, kt = tiles[b]["qt"], tiles[b]["kt"]
                if "ctx" not in tiles[b]:
                    ctx = bp.tile([128, S], F32R, tag="ctx", name=f"ctx{b}")
                    tiles[b]["ctx"] = ctx
                ctx = tiles[b]["ctx"]
                vns = (tiles[b]["vna"], tiles[b]["vnb"])
                for j in range(jlo, jhi) if "attn" in PHASES else []:
                    qsl = slice(j * QT_W, (j + 1) * QT_W)
                    acc = ps.tile([128, 512], F32, tag="acc", name=f"acc{b}_{j}",
                                  bufs=2)
                    nc.vector.memset(acc[:], 0.0)
                    nkc = 2 * (j + 1)              # causal: k-chunks 0..nkc-1
                    n_sc = (nkc + 3) // 4
                    for sc in range(n_sc):
                        cs = [c for c in range(4 * sc, min(4 * sc + 4, nkc))]
                        sts, pts = [], []
                        for h in range(2):
                            st_h = ps.tile([128, 4 * QT_W], F32, tag=f"st{h}",
                                           name=f"st{h}_{b}_{j}_{sc}")
                            pt_h = wp.tile([128, 4 * QT_W], F32R, tag=f"pt{h}",
                                           name=f"pt{h}_{b}_{j}_{sc}", bufs=5)
                            sts.append(st_h)
                            pts.append(pt_h)
                        for c in cs:   # QK: heads adjacent -> row-group concurrency
                            for h in range(2):
                                hsl = slice(h * HD, (h + 1) * HD)
                                nc.tensor.matmul(
                                    sts[h][:, (c - 4 * sc) * QT_W:(c - 4 * sc + 1) * QT_W],
                                    kt[hsl, c * KC:(c + 1) * KC],
                                    qt[hsl, qsl],
                                    start=True, stop=True,
                                )
                        w = len(cs) * QT_W
                        for h in range(2):
                            nc.scalar.activation(pts[h][:, 0:w], sts[h][:, 0:w],
                                                 AF.Exp, scale=float(SCALE))
                        if sc == n_sc - 1:  # diagonal: mask last two k-chunks
                            for h in range(2):
                                for c in (nkc - 2, nkc - 1):
                                    mo = 384 - 128 * (c - 2 * j)  # o = 128*(c-2j)
                                    nc.gpsimd.tensor_mul(
                                        pts[h][:, (c - 4 * sc) * QT_W:(c - 4 * sc + 1) * QT_W],
                                        pts[h][:, (c - 4 * sc) * QT_W:(c - 4 * sc + 1) * QT_W],
                                        mk_s[:, mo:mo + QT_W],
                                    )
                        for c in cs:   # P@V (+ones rowsum row)
                            for h in range(2):
                                nc.tensor.matmul(
                                    acc[0:HD + 1, h * QT_W:(h + 1) * QT_W],
                                    vns[h][:, c * (HD + 1):(c + 1) * (HD + 1)],
                                    pts[h][:, (c - 4 * sc) * QT_W:(c - 4 * sc + 1) * QT_W],
                                    start=False, stop=(c == nkc - 1),
                                    skip_group_check=True,
                                )
                    # normalize: one recip over both heads' rowsum halves,
                    # partition-broadcast on the (idle) gpsimd, one fused mul
                    recip = wp.tile([1, 2 * QT_W], F32, tag="recip",
                                    name=f"rc{b}_{j}")
                    nc.vector.reciprocal(recip[:], acc[HD:HD + 1, :])
                    for h in range(2):
                        asl = slice(h * QT_W, (h + 1) * QT_W)
                        bc_sb = wp.tile([HD, QT_W], F32, tag="bcs",
                                        name=f"bcs{b}_{j}_{h}", bufs=4)
                        nc.gpsimd.partition_broadcast(bc_sb[:], recip[0:1, asl])
                        nc.any.tensor_mul(ctx[h * HD:(h + 1) * HD, qsl],
                                          acc[0:HD, asl], bc_sb[:])

            def emit_oproj(b):
                ctx = tiles[b]["ctx"]
                for qp in range(S // 256) if "oproj" in PHASES else []:
                    osb = wp.tile([128, 2048], F32, tag="osb", name=f"ob{b}_{qp}")
                    for sub in range(2):
                        qc = 2 * qp + sub
                        for half in range(2):
                            osl = slice(half * 512, (half + 1) * 512)
                            op = pm.tile([128, 512], F32, tag="mix",
                                         name=f"op{b}_{qc}_{half}")
                            nc.tensor.matmul(op[:], ctx[:, qc * 128:(qc + 1) * 128],
                                             wo_s[:, osl], start=True, stop=True)
                            nc.vector.tensor_copy(
                                osb[:, sub * 1024 + half * 512:
                                    sub * 1024 + (half + 1) * 512], op[:])
                    nc.sync.dma_start(
                        out_d.ap()[b, qp * 256:(qp + 1) * 256, :]
                        .rearrange("(g q) o -> q g o", g=2),
                        osb.rearrange("p (g o) -> p g o", g=2))

            # software-pipelined emission: batch b+1's projection halves are
            # interleaved into batch b's (ACT-gated) attention j-loop so PE
            # always has prioritized fill work; the heavier fill (half 1 +
            # V-transpose) lands before the large causal j-tiles
            emit_proj(0)
            emit_vtrans(0)
            for b in range(B):
                if b + 1 < B:
                    emit_proj(b + 1, halves=(0,))
                emit_attn(b, 0, 4)
                if b + 1 < B:
                    emit_proj(b + 1, halves=(1,))
                    emit_vtrans(b + 1)
                emit_attn(b, 4, N_QT)
                emit_oproj(b)

                # ---- output projection (partial over this core's channels) ----

    nc.compile()
    return nc


def _get_nc():
    if "nc" not in _CACHE:
        _CACHE["nc"] = _build_nc()
    return _CACHE["nc"]


def make_in_maps(x, Wq, bq, Wk, bk, Wv, bv, Wo):
    """Host-side sharding: returns per-core input dicts."""
    xt = np.ascontiguousarray(np.transpose(np.asarray(x, np.float32), (0, 2, 1)))
    mask = (np.arange(896, dtype=np.int64)[None, :]
            >= (np.arange(128, dtype=np.int64)[:, None] + 384)).astype(np.float32)
    ident = np.eye(128, dtype=np.float32)
    ones16 = np.ones((128, N_KC), dtype=np.float32)
    in_maps = []
    for i in range(NCORES):
        r = slice(i * C, (i + 1) * C)
        in_maps.append({
            "xt": xt,
            "wqt": np.ascontiguousarray(np.asarray(Wq, np.float32)[r, :].T),
            "wkt": np.ascontiguousarray(np.asarray(Wk, np.float32)[r, :].T),
            "wvt": np.ascontiguousarray(np.asarray(Wv, np.float32)[r, :].T),
            "wot": np.ascontiguousarray(np.asarray(Wo, np.float32)[:, r].T),
            "bq": np.asarray(bq, np.float32)[r].reshape(C, 1),
            "bk": np.asarray(bk, np.float32)[r].reshape(C, 1),
            "bv": np.asarray(bv, np.float32)[r].reshape(C, 1),
            "maskbuf": mask,
            "ident": ident,
            "ones16": ones16,
        })
    return in_maps


def run_cores(in_maps):
    nc = _get_nc()
    res = run_bass_kernel_spmd(nc, in_maps, core_ids=list(range(NCORES)))
    return [r["out"] for r in res.results]


def kernel(x, mask, Wq, bq, Wk, bk, Wv, bv, Wo, bo):
    in_maps = make_in_maps(x, Wq, bq, Wk, bk, Wv, bv, Wo)
    partials = run_cores(in_maps)
    out = partials[0]
    for p in partials[1:]:
        out = out + p
    return (out + np.asarray(bo, np.float32)[None, None, :]).astype(np.float32)



# revision 9
# speedup vs baseline: 1.0823x; 1.0823x over previous
"""Multi-head causal attention (B=4, S=2048, H=1024, NH=16) on 8 trn2 cores.

Head-sharded tensor parallelism: core i computes heads {2i, 2i+1} (channel
slice r = [128i, 128i+128)).  Numerics plan (rel-err budget 2e-2, measured
~1.4e-2 in numpy emulation):
  - Q/K projections: fp8(e4m3) DoubleRow matmuls over host-packed pair
    layouts; weights prescaled by 32 to dodge e4m3 subnormals (1/32**2
    folded into the exp scale).
  - qt/kt re-quantized to fp8 and pair-packed (d-pairs) via a DRAM
    round-trip DMA; QK scores run as fp8 DoubleRow (0.5 cycles/row).
  - softmax: exp on ACT (psum f32 -> bf16 P), causal masking via two
    [128,128] triangle tensor-tensor multiplies per q-tile; the fully
    masked half of the last diagonal k-chunk is never computed.
  - V projection in bf16 directly in [s, d] layout (kills the V transpose);
    V bias folded into the output bias on the host (bo' = bo + Wo @ bv);
    K bias dropped entirely (softmax is shift-invariant per query).
  - P@V in bf16 with a ones column (rowsum rides row 64 of the psum acc);
    normalize via reciprocal + gpsimd partition-broadcast fused into the
    ctx eviction multiply.
  - output projection bf16; partial outputs written as bf16, all-reduced
    (summed) on the host.
"""
import numpy as np
import ml_dtypes

import concourse.bacc as bacc
import concourse.tile as tile
from concourse import mybir
from concourse.bass_utils import run_bass_kernel_spmd

F32 = mybir.dt.float32
BF16 = mybir.dt.bfloat16
FP8 = mybir.dt.float8e4
AF = mybir.ActivationFunctionType
DR = mybir.MatmulPerfMode.DoubleRow
MULT = mybir.AluOpType.mult

B, S, H, NH = 4, 2048, 1024, 16
HD = H // NH            # 64
NCORES = 8
C = 128                 # channels per core (2 heads)
SW = 32.0               # fp8 weight prescale
EXP_SCALE = (1.0 / np.sqrt(HD)) / (SW * SW)

QT_W = 256              # q-tile width
KC = 128                # k-chunk
N_QT = S // QT_W        # 8
N_KC = S // KC          # 16
GW = 4                  # k-chunks per ST group
VG = 130                # vt column group: 64 d (h0) | 1 | 64 d (h1) | 1

_CACHE = {}


def _build_nc():
    nc = bacc.Bacc(name="mha_tp8")
    xbt_d = nc.dram_tensor("xbt", [B, H, S], BF16, kind="ExternalInput")
    x8p_d = nc.dram_tensor("x8p", [B, 4, 128, 2 * S], FP8, kind="ExternalInput")
    wq8_d = nc.dram_tensor("wq8", [4, 128, 2 * 128], FP8, kind="ExternalInput")
    wk8_d = nc.dram_tensor("wk8", [4, 128, 2 * 128], FP8, kind="ExternalInput")
    wvt_d = nc.dram_tensor("wvt", [H, C], BF16, kind="ExternalInput")
    wot_d = nc.dram_tensor("wot", [C, H], BF16, kind="ExternalInput")
    bqs_d = nc.dram_tensor("bqs", [C, 1], F32, kind="ExternalInput")
    tri_d = nc.dram_tensor("trib", [128, 256], BF16, kind="ExternalInput")
    scr_d = nc.dram_tensor("qscr", [B, 2, 128, S], FP8, kind="Internal")
    out_d = nc.dram_tensor("out", [B, S, H], BF16, kind="ExternalOutput")

    with tile.TileContext(nc) as tc:
        with (
            tc.tile_pool(name="const", bufs=1) as cp,
            tc.tile_pool(name="xb", bufs=2) as xbp,
            tc.tile_pool(name="x8", bufs=2) as x8pool,
            tc.tile_pool(name="qk", bufs=2) as qkp,
            tc.tile_pool(name="big", bufs=2) as bp,
            tc.tile_pool(name="wk", bufs=4) as wp,
            tc.tile_pool(name="st", bufs=1, space="PSUM") as stp,
            tc.tile_pool(name="acc", bufs=2, space="PSUM") as accp,
            tc.tile_pool(name="mix", bufs=2, space="PSUM") as mixp,
        ):
            # ---- constants ----
            wq8_s = cp.tile([128, 4 * 256], FP8)
            wk8_s = cp.tile([128, 4 * 256], FP8)
            wvt_s = cp.tile([128, 8 * 128], BF16)
            wo_s = cp.tile([128, H], BF16)
            tri_s = cp.tile([128, 256], BF16)
            bq_s = cp.tile([C, 1], F32)
            nc.sync.dma_start(
                wq8_s.rearrange("p (kc m) -> p kc m", kc=4),
                wq8_d.ap().rearrange("kc p m -> p kc m"))
            nc.sync.dma_start(
                wk8_s.rearrange("p (kc m) -> p kc m", kc=4),
                wk8_d.ap().rearrange("kc p m -> p kc m"))
            nc.sync.dma_start(
                wvt_s.rearrange("p (hc c) -> p hc c", hc=8),
                wvt_d.ap().rearrange("(hc p) c -> p hc c", p=128))
            nc.sync.dma_start(wo_s[:], wot_d.ap())
            nc.sync.dma_start(tri_s[:], tri_d.ap())
            nc.sync.dma_start(bq_s[:], bqs_d.ap())

            tiles = {}

            def emit_loads(b):
                t = {}
                t["xb"] = []
                for hc in range(8):
                    xt = xbp.tile([128, S], BF16, tag=f"xb{hc}", name=f"xb{b}_{hc}")
                    nc.sync.dma_start(xt[:], xbt_d.ap()[b, hc * 128:(hc + 1) * 128, :])
                    t["xb"].append(xt)
                t["x8"] = []
                for kc in range(4):
                    x8t = x8pool.tile([128, 2 * S], FP8, tag=f"x8{kc}",
                                      name=f"x8{b}_{kc}")
                    nc.sync.dma_start(x8t[:], x8p_d.ap()[b, kc, :, :])
                    t["x8"].append(x8t)
                tiles[b] = t

            def emit_qkproj(b, sts):
                """fp8 DoubleRow Q/K projections for st groups in sts."""
                t = tiles[b]
                if "qt8" not in t:
                    t["qt8"] = qkp.tile([128, S], FP8, tag="qt8", name=f"qt8_{b}")
                    t["kt8"] = qkp.tile([128, S], FP8, tag="kt8", name=f"kt8_{b}")
                for st in sts:
                    sl = slice(st * 512, (st + 1) * 512)
                    for w_s, dst, bias, nm in ((wq8_s, t["qt8"], bq_s, "q"),
                                               (wk8_s, t["kt8"], None, "k")):
                        pp = mixp.tile([128, 512], F32, tag="mix",
                                       name=f"pp{nm}{b}_{st}")
                        for kc in range(4):
                            nc.tensor.matmul(
                                pp[:],
                                w_s.rearrange("p (kc two m) -> p kc two m",
                                              kc=4, two=2)[:, kc],
                                t["x8"][kc].rearrange("p (two s) -> p two s",
                                                      two=2)[:, :, sl],
                                start=(kc == 0), stop=(kc == 3), perf_mode=DR)
                        if bias is not None:
                            nc.vector.tensor_scalar_add(dst[:, sl], pp[:], bias[:])
                        else:
                            nc.vector.tensor_copy(dst[:, sl], pp[:])

            def emit_pair_dma(b, which=(0, 1)):
                """qt8/kt8 -> DRAM -> d-pair layout [32, (h two s)]."""
                t = tiles[b]
                for i in which:
                    nat = (t["qt8"], t["kt8"])[i]
                    pr = qkp.tile([32, 4 * S], FP8, tag=f"pr{i}", name=f"pr{i}_{b}")
                    nc.sync.dma_start(scr_d.ap()[b, i], nat[:])
                    nc.sync.dma_start(
                        pr.rearrange("p (h two s) -> p h two s", h=2, two=2),
                        scr_d.ap()[b, i].rearrange("(h p two) s -> p h two s",
                                                   h=2, two=2))
                    t["qp8" if i == 0 else "kp8"] = pr

            def emit_vpass(b, quads=(0, 1, 2, 3)):
                """V projection in [s, d] layout, bf16, plus the ones cols."""
                t = tiles[b]
                if "vt" not in t:
                    t["vt"] = bp.tile([128, N_KC * VG], BF16, tag="vt",
                                      name=f"vt{b}")
                    # ones columns at offsets c*VG+64 and c*VG+129
                    nc.gpsimd.memset(
                        t["vt"].rearrange("p (c g) -> p c g", g=VG // 2)
                        [:, :, 64:65], 1.0)
                vt = t["vt"]
                for q4 in quads:
                    pv = mixp.tile([128, 512], F32, tag="mix", name=f"pv{b}_{q4}")
                    for sc in range(4):
                        c = q4 * 4 + sc
                        csl = slice(c * 128, (c + 1) * 128)
                        for hc in range(8):
                            nc.tensor.matmul(
                                pv[:, sc * 128:(sc + 1) * 128],
                                t["xb"][hc][:, csl],
                                wvt_s[:, hc * 128:(hc + 1) * 128],
                                start=(hc == 0), stop=(hc == 7),
                                skip_group_check=True)
                    # dest: (c, h, d) -> col c*VG + h*65 + d
                    nc.vector.tensor_copy(
                        vt.rearrange("p (c h g) -> p c h g", h=2, g=VG // 2)
                        [:, 4 * q4:4 * q4 + 4, :, 0:64],
                        pv.rearrange("p (sc h d) -> p sc h d", sc=4, h=2))

            def emit_attn_j(b, j):
                t = tiles[b]
                if "ctx" not in t:
                    t["ctx"] = bp.tile([128, S], BF16, tag="ctx", name=f"ctx{b}")
                qp8, kp8, vt, ctx = t["qp8"], t["kp8"], t["vt"], t["ctx"]
                qp4 = qp8.rearrange("p (h two s) -> p h two s", h=2, two=2)
                kp4 = kp8.rearrange("p (h two s) -> p h two s", h=2, two=2)
                nkc = 2 * (j + 1)
                qsl = slice(j * QT_W, (j + 1) * QT_W)
                acc = accp.tile([65, 512], F32, tag="acc", name=f"acc{b}_{j}")
                n_g = (nkc + GW - 1) // GW
                pts_tiles = []
                for g in range(n_g):
                    cs = list(range(GW * g, min(GW * g + GW, nkc)))
                    w = len(cs)
                    last_g = g == n_g - 1
                    st = stp.tile([128, 2048], F32, tag="st",
                                  name=f"st{b}_{j}_{g}")
                    pts = wp.tile([128, 2048], BF16, tag="pts",
                                  name=f"pts{b}_{j}_{g}")
                    # ---- QK fp8 DoubleRow (transposed scores S^T[k, q]) ----
                    for h in range(2):
                        for ci, c in enumerate(cs):
                            if c == nkc - 1:
                                # diagonal: first 128 q-cols fully masked
                                nc.tensor.matmul(
                                    st[:, h * 1024 + ci * 256 + 128:
                                       h * 1024 + ci * 256 + 256],
                                    kp4[:, h, :, c * KC:(c + 1) * KC],
                                    qp4[:, h, :, j * QT_W + 128:(j + 1) * QT_W],
                                    start=True, stop=True, perf_mode=DR)
                            else:
                                nc.tensor.matmul(
                                    st[:, h * 1024 + ci * 256:
                                       h * 1024 + (ci + 1) * 256],
                                    kp4[:, h, :, c * KC:(c + 1) * KC],
                                    qp4[:, h, :, qsl],
                                    start=True, stop=True, perf_mode=DR)
                    # ---- exp (both heads, one call; last group splits to
                    # skip the never-written half of the diagonal chunk) ----
                    if last_g:
                        if w > 1:
                            nc.scalar.activation(
                                pts.rearrange("p (h c) -> p h c", h=2)
                                [:, :, 0:(w - 1) * 256],
                                st.rearrange("p (h c) -> p h c", h=2)
                                [:, :, 0:(w - 1) * 256],
                                AF.Exp, scale=float(EXP_SCALE))
                        nc.scalar.activation(
                            pts.rearrange("p (h c) -> p h c", h=2)
                            [:, :, (w - 1) * 256 + 128:w * 256],
                            st.rearrange("p (h c) -> p h c", h=2)
                            [:, :, (w - 1) * 256 + 128:w * 256],
                            AF.Exp, scale=float(EXP_SCALE))
                    else:
                        nc.scalar.activation(
                            pts.rearrange("p (h c) -> p h c", h=2)[:, :, 0:w * 256],
                            st.rearrange("p (h c) -> p h c", h=2)[:, :, 0:w * 256],
                            AF.Exp, scale=float(EXP_SCALE))
                    if last_g:
                        # ---- causal triangle masks on the 2 diagonal chunks ----
                        s0 = (w - 2) * 2  # 128-col segment of chunk nkc-2
                        for h in range(2):
                            seg = pts.rearrange("p (a c) -> p a c", c=128)
                            a0 = h * 8 + s0
                            nc.gpsimd.tensor_tensor(
                                seg[:, a0:a0 + 4:3, :], seg[:, a0:a0 + 4:3, :],
                                tri_s.rearrange("p (a c) -> p a c", c=128),
                                MULT)
                    pts_tiles.append(pts)
                # ---- P@V (bf16, ones col rides as acc row 64) ----
                # PSUM pending-zero regions are whole-bank (2KB): the two
                # heads' accumulation chains share acc's bank, so head 0's
                # chain must fully stop before head 1's start re-marks the
                # bank. Emit all of h0's chunks, then all of h1's.
                for h in range(2):
                    for c in range(nkc):
                        pts = pts_tiles[c // GW]
                        ci = c % GW
                        lo = 128 if c == nkc - 1 else 0
                        nc.tensor.matmul(
                            acc[:, h * 256 + lo:(h + 1) * 256],
                            vt[:, c * VG + h * 65:c * VG + h * 65 + 65],
                            pts[:, h * 1024 + ci * 256 + lo:
                                h * 1024 + (ci + 1) * 256],
                            start=(c == 0), stop=(c == nkc - 1),
                            skip_group_check=True)
                # ---- normalize: recip + partition-broadcast + fused evict ----
                rcp = wp.tile([1, 512], F32, tag="rcp", name=f"rcp{b}_{j}")
                nc.vector.reciprocal(rcp[:], acc[64:65, :])
                for h in range(2):
                    bc = wp.tile([64, 256], F32, tag="bc", name=f"bc{b}_{j}_{h}")
                    nc.gpsimd.partition_broadcast(
                        bc[:], rcp[0:1, h * 256:(h + 1) * 256])
                    nc.vector.tensor_tensor(
                        ctx[h * 64:(h + 1) * 64, qsl],
                        acc[0:64, h * 256:(h + 1) * 256], bc[:], MULT)

            def emit_oproj(b):
                t = tiles[b]
                ctx = t["ctx"]
                for qc in range(16):
                    osb = wp.tile([128, 1024], BF16, tag="osb",
                                  name=f"osb{b}_{qc}", bufs=3)
                    for half in range(2):
                        po = mixp.tile([128, 512], F32, tag="mix",
                                       name=f"po{b}_{qc}_{half}")
                        nc.tensor.matmul(po[:], ctx[:, qc * 128:(qc + 1) * 128],
                                         wo_s[:, half * 512:(half + 1) * 512],
                                         start=True, stop=True)
                        nc.vector.tensor_copy(
                            osb[:, half * 512:(half + 1) * 512], po[:])
                    nc.sync.dma_start(out_d.ap()[b, qc * 128:(qc + 1) * 128, :],
                                      osb[:])

            # ---- software-pipelined emission ----
            emit_loads(0)
            emit_qkproj(0, (0, 1, 2, 3))
            emit_pair_dma(0)
            emit_vpass(0)
            for b in range(B):
                nb = b + 1
                for j in range(N_QT):
                    emit_attn_j(b, j)
                    if nb < B:
                        if j == 0:
                            emit_loads(nb)
                        elif j == 1:
                            emit_qkproj(nb, (0, 1))
                        elif j == 2:
                            emit_qkproj(nb, (2, 3))
                        elif j == 3:
                            emit_pair_dma(nb)
                        elif j in (4, 5, 6, 7):
                            emit_vpass(nb, (j - 4,))
                emit_oproj(b)

    nc.compile()
    return nc


def _get_nc():
    if "nc" not in _CACHE:
        _CACHE["nc"] = _build_nc()
    return _CACHE["nc"]


def make_in_maps(x, Wq, bq, Wk, bk, Wv, bv, Wo):
    """Host-side sharding: returns per-core input dicts."""
    E4 = ml_dtypes.float8_e4m3
    BF = ml_dtypes.bfloat16
    f32 = np.float32
    x = np.asarray(x, f32)
    xbt = np.ascontiguousarray(x.transpose(0, 2, 1).astype(BF))    # [B, H, S]
    x8t = x.astype(E4)                                              # [B, S, H]
    # pairs: x8p[b, kc, p, i, s] = fp8(x[b, s, 256kc + 2p + i])
    x8p = np.ascontiguousarray(
        x8t.transpose(0, 2, 1).reshape(B, 4, 128, 2, S).reshape(B, 4, 128, 2 * S))
    tri = np.triu(np.ones((128, 128), f32))
    trib = np.ascontiguousarray(np.concatenate([tri, tri], axis=1).astype(BF))

    def wpack(W, r):
        # [4, 128, 2, 128]: (kc, p, i, m) = SW * W[r0+m, 256kc + 2p + i]
        wt = (SW * np.asarray(W, f32)[r, :]).astype(E4)   # [128 out, 1024 in]
        wt = wt.T.reshape(4, 128, 2, 128)                  # in-major pairs
        return np.ascontiguousarray(wt.reshape(4, 128, 2 * 128))

    in_maps = []
    for i in range(NCORES):
        r = slice(i * C, (i + 1) * C)
        in_maps.append({
            "xbt": xbt,
            "x8p": x8p,
            "wq8": wpack(Wq, r),
            "wk8": wpack(Wk, r),
            "wvt": np.ascontiguousarray(np.asarray(Wv, f32)[r, :].T.astype(BF)),
            "wot": np.ascontiguousarray(np.asarray(Wo, f32)[:, r].T.astype(BF)),
            "bqs": (SW * np.asarray(bq, f32)[r]).reshape(C, 1),
            "trib": trib,
        })
    return in_maps


def run_cores(in_maps):
    nc = _get_nc()
    res = run_bass_kernel_spmd(nc, in_maps, core_ids=list(range(NCORES)))
    return [r["out"] for r in res.results]


def kernel(x, mask, Wq, bq, Wk, bk, Wv, bv, Wo, bo):
    in_maps = make_in_maps(x, Wq, bq, Wk, bk, Wv, bv, Wo)
    partials = run_cores(in_maps)
    out = np.zeros((B, S, H), np.float32)
    for p in partials:
        out += np.asarray(p, dtype=np.float32)
    bo_p = (np.asarray(bo, np.float64)
            + np.asarray(Wo, np.float64) @ np.asarray(bv, np.float64))
    return (out + bo_p[None, None, :].astype(np.float32)).astype(np.float32)
